# revision 13
# baseline (speedup 1.0000x reference)
"""Trainium2 Bass kernel for DiffusionGraphConv (DCRNN-style graph diffusion).

Math (per reference):
  x0 = reshape(inputs) -> [N, P*B]
  for each of 2 sparse transition matrices A (COO, E edges):
     x1 = A @ x0 ;  x2 = 2*A@x1 - x0
  out = concat([x0, x1_a, x2_a, x1_b, x2_b]) @ weight + bias

Sharding: graph-split x batch-split.  Cores 0-3 handle transition
matrix 1, cores 4-7 matrix 2; core c and c+4 both hold batches
8c..8c+7 (F = 8*32 = 256 features per core).  Each core returns a
PARTIAL output [NPAD, 512] fp16 (x1/t terms of its graph); the host
sums the pair and adds the x0@(W0-W2-W4) term + bias itself (tiny
dense GEMM) during unshard.  No collectives.

SpMM mapping (dense-blocked -- no DMA gathers at all):
  A per-edge dma_gather formulation is capped ~59GB/s/core by GpSimd Q7
  descriptor generation.  Instead the host scatters A into dense fp16
  blocks S[j][w] = A^T[srctile j (128), dstwin w (512)] (multi-hot with
  values; ~0.2% nnz) and the device computes, per dst window w:
      y^T[f_half, w] += sum_j  x_j[128 src, f_half]^T @ S_jw[128, 512]
  streaming S from HBM on the HWDGE sync queue at full sequential
  bandwidth while the TensorEngine runs 512-col matmuls back-to-back
  (the 128x128 stationary x-tile LDW hides under the stream; PE stays
  at full p-state, measured ~100% busy; matmul PSUM output is capped
  at one bank = 512 f32, hence W=512).  y^T lands feature-major:
  phase 0 drains it into an x1T slab (+ PE transposes for the
  node-major x1 tiles phase 1 contracts); phase 1 projects x1/t per
  dst tile and stores per window.  x1 never round-trips through HBM;
  x0 ships pre-tiled so its load is one contiguous DMA.
"""

import sys

import numpy as np

sys.path.insert(0, "/opt/trn_rl_repo")

import concourse.bass as bass
import concourse.bacc as bacc
import concourse.mybir as mybir
import concourse.tile as tile
from concourse.bass_utils import run_bass_kernel_spmd

dt = mybir.dt

N, P, Q, B, E = 10000, 32, 64, 32, 160000
NT = 79              # 128-row node tiles
NPAD = NT * 128      # 10112
NB = 8               # batches per core
F = NB * P           # 256 features per core
FO = NB * Q          # 512 output features per core
NCORES = 8
W = 512              # dst window (S block cols; PSUM bank = 512 f32)
NW = -(-NPAD // W)   # 20 dst windows (last 384 wide, zero-padded)
WPAD = NW * W        # 10240
SGRP = 8             # S blocks (src tiles) per DMA


def _build_nc():
    nc = bacc.Bacc("TRN2", target_bir_lowering=False, debug=False,
                   num_devices=NCORES)

    x0nm = nc.declare_dram_parameter("x0nm", [128, NT * F], dt.float16,
                                     isOutput=False)
    S_d = nc.declare_dram_parameter("Svals", [128, NW * NT * W], dt.float16,
                                    isOutput=False)
    W6_d = nc.declare_dram_parameter("W6", [128, 4 * FO], dt.float16,
                                     isOutput=False)
    ident_d = nc.declare_dram_parameter("ident", [128, 128], dt.float16,
                                        isOutput=False)
    out_d = nc.declare_dram_parameter("out", [NPAD, FO], dt.float16,
                                      isOutput=True)

    with tile.TileContext(nc) as tc:
        with (
            tc.tile_pool(name="const", bufs=1) as constp,
            tc.tile_pool(name="slabs", bufs=1) as slabp,
            tc.tile_pool(name="spool", bufs=4) as sp,
            tc.tile_pool(name="tT", bufs=2) as tTp,
            tc.tile_pool(name="ost", bufs=2) as ostp,
            tc.tile_pool(name="ypsum", bufs=2, space="PSUM") as yps,
            tc.tile_pool(name="tpsum", bufs=2, space="PSUM") as tps,
            tc.tile_pool(name="ppsum", bufs=2, space="PSUM") as pps,
        ):
            W6t = constp.tile([128, 4 * FO], dt.float16, tag="w6")
            nc.sync.dma_start(W6t[:], W6_d[:, :])
            ident = constp.tile([128, 128], dt.float16, tag="ident")
            nc.sync.dma_start(ident[:], ident_d[:, :])
            x0t = slabp.tile([128, NT, F], dt.float16, tag="x0")
            nc.sync.dma_start(
                x0t[:, :, :],
                x0nm[:, :].rearrange("p (i f) -> p i f", f=F))

            x1t = slabp.tile([128, NT, F], dt.float16, tag="x1")
            x1Tt = slabp.tile([128, 2, WPAD], dt.float16, tag="x1T")

            out_view = out_d[:, :].rearrange("(i p) f -> p i f", p=128)

            for phase in range(2):
                xsrc = x0t if phase == 0 else x1t
                for w in range(NW):
                    yh = [yps.tile([128, W], dt.float32, tag=f"y{h}",
                                   name=f"yh{h}") for h in range(2)]
                    St = None
                    for j in range(NT):
                        if j % SGRP == 0:
                            St = sp.tile([128, SGRP, W], dt.float16, tag="S")
                            nj = min(SGRP, NT - j)
                            base = (w * NT + j) * W
                            nc.sync.dma_start(
                                St[:, :nj, :],
                                S_d[:, base:base + nj * W].rearrange(
                                    "p (i c) -> p i c", c=W))
                        for h in range(2):
                            nc.tensor.matmul(
                                yh[h][:],
                                lhsT=xsrc[:, j, h * 128:(h + 1) * 128],
                                rhs=St[:, j % SGRP, :],
                                start=(j == 0), stop=(j == NT - 1),
                                skip_group_check=True)
                    nt_w = min(NT, (w + 1) * (W // 128)) - w * (W // 128)
                    if phase == 0:
                        for h in range(2):
                            nc.vector.tensor_copy(
                                x1Tt[:, h, w * W:(w + 1) * W], yh[h][:])
                        for it in range(nt_w):
                            t = w * (W // 128) + it
                            for h in range(2):
                                tp = tps.tile([128, 128], dt.float16,
                                              tag="tp")
                                nc.tensor.transpose(
                                    tp[:],
                                    x1Tt[:, h, t * 128:(t + 1) * 128],
                                    ident[:])
                                nc.vector.tensor_copy(
                                    x1t[:, t, h * 128:(h + 1) * 128], tp[:])
                    else:
                        tT = tTp.tile([128, 2, W], dt.float16, tag="tT")
                        for h in range(2):
                            nc.vector.tensor_copy(tT[:, h, :], yh[h][:])
                        ost = ostp.tile([128, W // 128, FO], dt.float16, tag="ost")
                        for it in range(nt_w):
                            t = w * (W // 128) + it
                            pp = pps.tile([128, FO], dt.float32, tag="pp")
                            for h in range(2):
                                nc.tensor.matmul(
                                    pp[:],
                                    lhsT=x1Tt[:, h, t * 128:(t + 1) * 128],
                                    rhs=W6t[:, h * FO:(h + 1) * FO],
                                    start=(h == 0), stop=False,
                                    skip_group_check=True)
                                nc.tensor.matmul(
                                    pp[:],
                                    lhsT=tT[:, h, it * 128:(it + 1) * 128],
                                    rhs=W6t[:, (2 + h) * FO:(3 + h) * FO],
                                    start=False, stop=(h == 1),
                                    skip_group_check=True)
                            nc.vector.tensor_copy(ost[:, it, :], pp[:])
                        nc.sync.dma_start(
                            out_view[:, w * (W // 128):w * (W // 128) + nt_w, :],
                            ost[:, :nt_w, :])
    nc.compile()
    return nc


def kernel(inputs, trans1_idx, trans1_val, trans2_idx, trans2_val,
           weight, bias):
    inputs = np.asarray(inputs, np.float32)
    weight = np.asarray(weight, np.float32)
    bias = np.asarray(bias, np.float32)

    # dense S blocks: S[p, (w*NT + j)*W + c] = sum of vals of edges
    # (dst = w*W + c) <- (src = j*128 + p);  duplicate edges must ADD
    Ss = []
    for idx, val in ((trans1_idx, trans1_val), (trans2_idx, trans2_val)):
        dst = np.asarray(idx[0]).astype(np.int64)
        src = np.asarray(idx[1]).astype(np.int64)
        v = np.asarray(val).astype(np.float32)
        S = np.zeros((128, NW * NT * W), np.float32)
        j, p = src // 128, src % 128
        w, c = dst // W, dst % W
        np.add.at(S, (p, (w * NT + j) * W + c), v)
        Ss.append(S.astype(np.float16))

    W_ = weight.reshape(P, 5, Q)
    w_ = [W_[:, m, :] for m in range(5)]
    terms = [[w_[1], 2 * w_[2]], [w_[3], 2 * w_[4]]]
    W6s = []
    for g in range(2):
        W6 = np.zeros((128, 4 * FO), np.float16)
        for m in range(2):
            for h in range(2):
                blk = np.zeros((128, FO), np.float32)
                for bl in range(4 * h, 4 * h + 4):
                    blk[(bl - 4 * h) * 32:(bl - 4 * h + 1) * 32,
                        bl * 64:(bl + 1) * 64] = terms[g][m]
                W6[:, (2 * m + h) * FO:(2 * m + h + 1) * FO] = blk
        W6s.append(W6)
    ident = np.eye(128, dtype=np.float16)

    T0 = (w_[0] - w_[2] - w_[4]).astype(np.float32)       # x0-term, host
    x0term = np.matmul(inputs.reshape(B, N, P), T0)       # [B, N, Q]

    in_maps = []
    for core in range(NCORES):
        g = core // 4
        cb = core % 4
        x0 = np.zeros((NPAD, F), np.float16)
        for bl in range(NB):
            x0[:N, bl * P:(bl + 1) * P] = \
                inputs[NB * cb + bl].reshape(N, P)
        x0til = np.ascontiguousarray(
            x0.reshape(NT, 128, F).transpose(1, 0, 2).reshape(128, NT * F))
        in_maps.append({
            "x0nm": x0til, "Svals": Ss[g], "W6": W6s[g], "ident": ident,
        })

    nc = _build_nc()
    res = run_bass_kernel_spmd(nc, in_maps, core_ids=list(range(NCORES)))

    out = np.empty((B, N * Q), np.float32)
    brow = np.tile(bias, NB)[None, :]
    for cb in range(4):
        pa = res.results[cb]["out"][:N].astype(np.float32)
        pb = res.results[cb + 4]["out"][:N].astype(np.float32)
        s = pa + pb + brow                       # [N, FO]
        for bl in range(NB):
            out[NB * cb + bl] = (s[:, bl * Q:(bl + 1) * Q]
                                 + x0term[NB * cb + bl]).reshape(N * Q)
    return out


if __name__ == "__main__":
    import reference
    inp = {k: np.asarray(v) for k, v in reference.setup_inputs().items()}
    expected = np.asarray(reference.reference(**inp))
    actual = kernel(**inp)
    rel = np.linalg.norm(actual - expected) / np.linalg.norm(expected)
    print("rel l2 err:", rel)
